# revision 11
# baseline (speedup 1.0000x reference)
"""Causal attention (B=4, S=4096, D_IN=768, D_OUT=64) on 8 Trainium2 NeuronCores.

Sharding: core c handles batch b=c//2 and key-parity p=c%2 (the even or odd
128-wide key tiles of that batch). Every core computes, for ALL queries of its
batch, the unnormalized attention partials over its own key set:
    num[o, q] = sum_{k in own} exp(q.k/8) * V[k, o]
    den[q]    = sum_{k in own} exp(q.k/8)
The host sums the two partials per batch and normalizes: ctx = (num/den).T.
Causality is exact: key-tile work is skipped below the diagonal band and the
two boundary blocks are masked with host-provided mask tiles.

Host prep per core: x[b].T in bf16, columns permuted to layout
[own0..own3, oth3, oth2, oth1, oth0] (512-col blocks), processed as
T = 0,1,2,3,7,6,5,4. The x stream is split across two DMA queues (sync:
blocks 0-3, gpsimd software-DGE: blocks 4-7) so delivery finishes ~2x
sooner; big tiles sit mid-kernel (their exps overlap later PE work) and
the smallest tile is last, keeping the exp-bound pipeline tail short.

All matmul operands are bf16 (walrus forbids mixing f32/f32r with 16-bit):
x, W, Q, K, V1, exp tiles. Accumulation stays fp32 in PSUM; host-measured
rel err ~3.3e-3 against the fp32 reference (tolerance 2e-2). bf16 moving
operands stream 1 col/cycle at any contraction size, so Q/K run unpadded
K=64 and the small diagonal-band matmuls avoid the f32r narrow-moving 4x
penalty.
"""
import numpy as np
import ml_dtypes

import concourse.bass as bass
import concourse.bacc as bacc
import concourse.tile as tile
from concourse import mybir
from concourse.bass_utils import run_bass_kernel_spmd

B, S, DI, DO = 4, 4096, 768, 64
NCORES = 8
NIC = DI // 128          # 6 contraction chunks
NKT = S // 128           # 32 global key tiles per batch
NOWN = NKT // 2          # 16 own key tiles per core
QT = 512                 # query tile width (one PSUM bank of fp32)
NQT = S // QT            # 8 query tiles
F32 = mybir.dt.float32
F32R = mybir.dt.float32r
BF16 = mybir.dt.bfloat16

ORDER = [0, 1, 2, 3, 7, 6, 5, 4]   # big tiles mid-kernel, smallest last
# layout block of each query tile (layout [own0..own3, oth3, oth2, oth1, oth0])
LBLK = {0: 0, 1: 1, 2: 2, 3: 3, 7: 4, 6: 5, 5: 6, 4: 7}
OWNBLK = [0, 1, 2, 3]              # layout block holding own key block st

_prog_cache = {}


def lblk(T):
    """Layout column-block (512 wide) of query tile T."""
    return LBLK[T]


def j0_of(T):
    """First diagonal-region key tile for query tile T."""
    return 4 * T if T < 4 else 4 * (T - 4)


def build_program():
    """Build + compile the single SPMD Bass program (identical on all cores)."""
    nc = bacc.Bacc("TRN2", target_bir_lowering=False, debug=False)

    xT = nc.declare_dram_parameter("xT", [DI, S], BF16, isOutput=False)
    wkv = nc.declare_dram_parameter("wkv", [DI, 128], BF16, isOutput=False)
    # Unpadded Wq: bf16 matmuls stream 1 col/cycle at any contraction
    # size, so scores run K=64 directly.
    wqp = nc.declare_dram_parameter("wqp", [DI, DO], BF16, isOutput=False)
    mdiag = nc.declare_dram_parameter("mdiag", [128, 128], BF16, isOutput=False)
    mpcol = nc.declare_dram_parameter("mpcol", [128, 128], BF16, isOutput=False)
    ident = nc.declare_dram_parameter("ident", [DO, DO], F32R, isOutput=False)
    nd = nc.declare_dram_parameter("nd", [DO + 1, S], F32, isOutput=True)

    with tile.TileContext(nc) as tc:
        with tc.tile_pool(name="consts", bufs=1) as consts, \
             tc.tile_pool(name="xpool", bufs=1) as xpool, \
             tc.tile_pool(name="qkv", bufs=1) as qkv, \
             tc.tile_pool(name="expp", bufs=8) as expp, \
             tc.tile_pool(name="ndst", bufs=2) as ndst, \
             tc.tile_pool(name="ps_proj", bufs=2, space="PSUM") as ps_proj, \
             tc.tile_pool(name="ps_sc2", bufs=2, space="PSUM") as ps_sc2, \
             tc.tile_pool(name="ps_ctx", bufs=2, space="PSUM") as ps_ctx:

            # ---- constants; first projection weights first so PE starts ASAP
            twkv = consts.tile([128, NIC, 128], BF16, tag="twkv", name="twkv")
            twq = consts.tile([128, NIC, DO], BF16, tag="twq", name="twq")
            nc.sync.dma_start(out=twkv, in_=wkv.rearrange("(c p) w -> p c w", p=128))

            # x^T layout blocks: 0 and 1 as [128, 512] singles (fast first
            # arrival), then pairs [128, 1024]; arrival order == consumption.
            xs = [[xpool.tile([128, QT], BF16, tag=f"xs_{ic}_{lb}", name=f"xs_{ic}_{lb}")
                   for lb in range(2)] for ic in range(NIC)]
            xcp = [[xpool.tile([128, 2 * QT], BF16, tag=f"xcp_{ic}_{p}", name=f"xcp_{ic}_{p}")
                    for p in range(1, 4)] for ic in range(NIC)]
            for ic in range(NIC):
                nc.sync.dma_start(out=xs[ic][0],
                                  in_=xT[ic * 128:(ic + 1) * 128, 0:QT])

            twq_dma = nc.sync.dma_start(out=twq,
                                        in_=wqp.rearrange("(c p) w -> p c w", p=128))
            tmd = consts.tile([128, 128], BF16, tag="tmd", name="tmd")
            tmp = consts.tile([128, 128], BF16, tag="tmp", name="tmp")
            tid = consts.tile([DO, DO], F32R, tag="tid", name="tid")
            nc.sync.dma_start(out=tmd, in_=mdiag[:, :])
            nc.sync.dma_start(out=tmp, in_=mpcol[:, :])
            nc.sync.dma_start(out=tid, in_=ident[:, :])
            zsrc = consts.tile([DO, 1], F32, tag="zsrc", name="zsrc")
            nc.vector.memset(zsrc, 0.0)
            ones = consts.tile([128, 1], BF16, tag="ones", name="ones")
            nc.vector.memset(ones, 1.0)

            # Dummy exp to pull the ~2.7us ACT table load off the critical path.
            zexp = consts.tile([DO, 1], F32, tag="zexp", name="zexp")
            nc.scalar.activation(zexp, zsrc,
                                 mybir.ActivationFunctionType.Exp, scale=1.0)

            for ic in range(NIC):
                nc.sync.dma_start(out=xs[ic][1],
                                  in_=xT[ic * 128:(ic + 1) * 128, QT:2 * QT])
            for ic in range(NIC):
                nc.sync.dma_start(
                    out=xcp[ic][0],
                    in_=xT[ic * 128:(ic + 1) * 128, 2 * QT:4 * QT])
            # late blocks ride the otherwise-idle gpsimd software-DGE queue,
            # halving x delivery time vs the serial sync queue
            for p in (2, 3):
                for ic in range(NIC):
                    nc.gpsimd.dma_start(
                        out=xcp[ic][p - 1],
                        in_=xT[ic * 128:(ic + 1) * 128,
                               p * 2 * QT:(p + 1) * 2 * QT])

            def xc(ic, lb):
                """[128, 512] view of layout column block lb."""
                if lb < 2:
                    return xs[ic][lb]
                return xcp[ic][lb // 2 - 1][:, (lb % 2) * QT:(lb % 2 + 1) * QT]

            # ---- projection helpers (emitted interleaved with attention) ----
            kts = [qkv.tile([DO, QT], BF16, tag=f"kt_{st}", name=f"kt_{st}") for st in range(4)]
            vts = [qkv.tile([DO, QT], F32R, tag=f"vt_{st}", name=f"vt_{st}") for st in range(4)]
            qts = [qkv.tile([DO, QT], BF16, tag=f"qt_{st}", name=f"qt_{st}") for st in range(NQT)]
            v1s = [qkv.tile([128, DO + 1], BF16, tag=f"v1_{j}", name=f"v1_{j}")
                   for j in range(NOWN)]

            def emit_pass1(st):
                """[K^T | V^T] matmuls over own key block st + kts/vts casts.

                The V1 transposes are NOT emitted here: they would chain
                PE->DVE->PE->DVE (transpose waits on the vts cast, the next
                scores wait on the v1/qts casts) and park the PE for ~2us per
                block. They are returned as thunks and interleaved into the
                next tile's scores stream, where the PE has slack while ACT
                runs exps.
                """
                p1 = ps_proj.tile([128, QT], F32, tag="psproj", name="psproj")
                for ic in range(NIC):
                    nc.tensor.matmul(p1, twkv[:, ic, :], xc(ic, OWNBLK[st]),
                                     start=(ic == 0), stop=(ic == NIC - 1))
                nc.vector.tensor_copy(kts[st], p1[0:DO, :])
                nc.vector.tensor_copy(vts[st], p1[DO:128, :])

            def transpose_job(j):
                """Build V1[j] ([128 keys, 65] bf16) from vts via PE transpose."""
                st, col = j // 4, (j % 4) * 128
                pv = ps_proj.tile([128, DO], F32R, tag="psproj", name="psproj")
                nc.tensor.transpose(pv, vts[st][:, col:col + 128], tid)
                nc.vector.tensor_copy(v1s[j][:, 0:DO], pv)
                # ones column for the row-sum (denominator)
                nc.vector.tensor_copy(v1s[j][:, DO:DO + 1], ones)

            def emit_pass2(T):
                """Q^T over query tile T (layout block lblk(T)); rows 64.. zero."""
                p2 = ps_proj.tile([DO, QT], F32, tag="psproj", name="psproj")
                for ic in range(NIC):
                    nc.tensor.matmul(p2, twq[:, ic, :], xc(ic, lblk(T)),
                                     start=(ic == 0), stop=(ic == NIC - 1))
                nc.vector.tensor_copy(qts[T], p2)

            # ---- attention: per query tile T, accumulate num/den over key tiles.
            exp_scale = float(1.0 / np.sqrt(DO))

            def emit_scores(T, j, sp_ap):
                """scores matmul for (T, j) into sp_ap ([128, w])."""
                r = j - j0_of(T)
                qlo = 128 * r if r > 0 else 0
                w = QT - qlo
                st, col = j // 4, (j % 4) * 128
                nc.tensor.matmul(sp_ap[:, 0:w], kts[st][:, col:col + 128],
                                 qts[T][:, qlo:QT], start=True, stop=True)
                return qlo, w

            class CtxDrain:
                """Phase B for a query tile, drained a couple of matmuls at a
                time between the NEXT tile's scores pairs so the in-order PE
                queue always holds ready work while ACT runs the exps."""

                def __init__(self, T, ctx_args):
                    self.T = T
                    self.nk = j0_of(T) + 4
                    self.args = ctx_args
                    self.i = 0
                    self.ctxp = ps_ctx.tile([DO + 1, QT], F32, tag="ctxp",
                                            name="ctxp")

                def drain(self, n):
                    while self.i < len(self.args) and n > 0:
                        j, et_ap, qlo, w = self.args[self.i]
                        nc.tensor.matmul(self.ctxp[:, qlo:QT], v1s[j],
                                         et_ap[:, 0:w],
                                         start=(j == 0), stop=(j == self.nk - 1))
                        self.i += 1
                        n -= 1

                def finish(self):
                    self.drain(len(self.args))
                    ost = ndst.tile([DO + 1, QT], F32, tag="ost", name="ost")
                    nc.vector.tensor_copy(ost, self.ctxp)
                    nc.sync.dma_start(
                        out=nd[:, lblk(self.T) * QT:(lblk(self.T) + 1) * QT],
                        in_=ost)

            emit_pass1(0)
            for j in range(4):
                transpose_job(j)   # startup is DMA-bound; no slack lost
            emit_pass2(0)
            pending = None     # CtxDrain from the previous iteration
            pending_tr = []    # deferred V1 transposes from the latest pass1
            for i, T in enumerate(ORDER):
                j0 = j0_of(T)
                mask = tmd if T < 4 else tmp
                ctx_args = []   # (j, et_ap, qlo, w) consumed in phase B
                for j in range(0, j0, 2):
                    sp2 = ps_sc2.tile([128, 2 * QT], F32, tag="sp2", name="sp2")
                    et2 = expp.tile([128, 2 * QT], BF16, tag="et", name="et")
                    emit_scores(T, j, sp2[:, 0:QT])
                    emit_scores(T, j + 1, sp2[:, QT:2 * QT])
                    nc.scalar.activation(et2, sp2,
                                         mybir.ActivationFunctionType.Exp,
                                         scale=exp_scale)
                    if pending_tr:
                        transpose_job(pending_tr.pop(0))
                    if pending is not None:
                        pending.drain(2)
                    ctx_args.append((j, et2[:, 0:QT], 0, QT))
                    ctx_args.append((j + 1, et2[:, QT:2 * QT], 0, QT))
                # diagonal band: r=0 (w=512) + r=1 (w=384) share a 2-bank tile;
                # r=2 (w=256) + r=3 (w=128) share a 1-bank tile
                spb1 = ps_sc2.tile([128, 2 * QT], F32, tag="sp2", name="sp2")
                etb1 = expp.tile([128, 2 * QT], BF16, tag="et", name="et")
                emit_scores(T, j0, spb1[:, 0:QT])
                emit_scores(T, j0 + 1, spb1[:, QT:QT + 384])
                nc.scalar.activation(etb1[:, 0:QT + 384], spb1[:, 0:QT + 384],
                                     mybir.ActivationFunctionType.Exp,
                                     scale=exp_scale)
                if pending_tr:
                    transpose_job(pending_tr.pop(0))
                if pending is not None:
                    pending.drain(2)
                nc.vector.tensor_mul(etb1[:, 0:128], etb1[:, 0:128], mask)
                nc.vector.tensor_mul(etb1[:, QT:QT + 128], etb1[:, QT:QT + 128], mask)
                ctx_args.append((j0, etb1[:, 0:QT], 0, QT))
                ctx_args.append((j0 + 1, etb1[:, QT:QT + 384], 128, 384))
                spb2 = ps_proj.tile([128, QT], F32, tag="psproj", name="psproj")
                etb2 = expp.tile([128, 2 * QT], BF16, tag="et", name="et")
                emit_scores(T, j0 + 2, spb2[:, 0:256])
                emit_scores(T, j0 + 3, spb2[:, 256:384])
                nc.scalar.activation(etb2[:, 0:384], spb2[:, 0:384],
                                     mybir.ActivationFunctionType.Exp,
                                     scale=exp_scale)
                if pending_tr:
                    transpose_job(pending_tr.pop(0))
                if pending is not None:
                    pending.drain(2)
                nc.vector.tensor_mul(etb2[:, 0:128], etb2[:, 0:128], mask)
                nc.vector.tensor_mul(etb2[:, 256:384], etb2[:, 256:384], mask)
                ctx_args.append((j0 + 2, etb2[:, 0:256], 256, 256))
                ctx_args.append((j0 + 3, etb2[:, 256:384], 384, 128))

                if pending is not None:
                    pending.finish()
                pending = CtxDrain(T, ctx_args)
                if i + 1 < NQT:
                    nxt = ORDER[i + 1]
                    if nxt < 4:
                        emit_pass1(nxt)
                        pending_tr = list(range(4 * nxt, 4 * nxt + 4))
                    emit_pass2(nxt)
            pending.finish()

    nc.compile()
    return nc


def get_program():
    if "nc" not in _prog_cache:
        _prog_cache["nc"] = build_program()
    return _prog_cache["nc"]


def core_perm(parity):
    """Layout-position -> global column index map.

    Layout = [own0, own1, own2, own3, oth3, oth2, oth1, oth0] 512-col blocks.
    """
    own = [g for g in range(NKT) if g % 2 == parity]
    other = [g for g in range(NKT) if g % 2 != parity]
    tiles = list(own)
    for b in (3, 2, 1, 0):
        tiles += other[4 * b:4 * b + 4]
    return np.concatenate([np.arange(g * 128, (g + 1) * 128) for g in tiles])


def make_in_maps(x, Wq, Wk, Wv):
    x = np.asarray(x, dtype=np.float32)
    Wq = np.asarray(Wq, dtype=np.float32)
    Wk = np.asarray(Wk, dtype=np.float32)
    Wv = np.asarray(Wv, dtype=np.float32)
    bf = ml_dtypes.bfloat16
    wkv = np.concatenate([Wk, Wv], axis=1).astype(bf)
    wqp = Wq.astype(bf)
    mdiag = np.triu(np.ones((128, 128), dtype=np.float32)).astype(bf)
    ident = np.eye(DO, dtype=np.float32)
    in_maps = []
    perms = []
    for c in range(NCORES):
        b, par = c // 2, c % 2
        perm = core_perm(par)
        perms.append(perm)
        xTp = np.ascontiguousarray(x[b].T[:, perm].astype(bf))
        mpcol = np.full((128, 128), 1.0 - par, dtype=np.float32).astype(bf)
        in_maps.append({
            "xT": xTp, "wkv": wkv, "wqp": wqp,
            "mdiag": mdiag, "mpcol": mpcol, "ident": ident,
        })
    return in_maps, perms


def combine(results, perms):
    out = np.empty((B, S, DO), dtype=np.float32)
    for b in range(B):
        num = np.zeros((DO, S), dtype=np.float64)
        den = np.zeros((S,), dtype=np.float64)
        for c in (2 * b, 2 * b + 1):
            nd_c = results[c]["nd"].astype(np.float64)
            inv = np.empty(S, dtype=np.int64)
            inv[perms[c]] = np.arange(S)
            nd_g = nd_c[:, inv]
            num += nd_g[:DO]
            den += nd_g[DO]
        out[b] = (num / den).T.astype(np.float32)
    return out


def kernel(x, Wq, Wk, Wv):
    nc = get_program()
    in_maps, perms = make_in_maps(x, Wq, Wk, Wv)
    res = run_bass_kernel_spmd(nc, in_maps, list(range(NCORES)))
    return combine(res.results, perms)


# revision 12
# speedup vs baseline: 1.0969x; 1.0969x over previous
"""Causal attention (B=4, S=4096, D_IN=768, D_OUT=64) on 8 Trainium2 NeuronCores.

Sharding: core c handles batch b=c//2 and key-parity p=c%2 (the even or odd
128-wide key tiles of that batch). Every core computes, for ALL queries of its
batch, the unnormalized attention partials over its own key set:
    num[o, q] = sum_{k in own} exp(q.k/8) * V[k, o]
    den[q]    = sum_{k in own} exp(q.k/8)
The host sums the two partials per batch and normalizes: ctx = (num/den).T.
Causality is exact: key-tile work is skipped below the diagonal band and the
two boundary blocks are masked with host-provided mask tiles.

Host prep per core: x[b].T in bf16, columns permuted to layout
[own0..own3, oth3, oth2, oth1, oth0] (512-col blocks). The x stream is
DMA-serial (~3.3us per block after a ~7.2us engine preamble), so tiles
are processed in arrival order T = 1,2,3,7,6,5 with the two nk=4 tiles
(T=0, T=4) deferred to the end: they only need kts[st0] plus their own
(long-resident) q-block, and their small exps keep the ACT-bound
pipeline tail to ~3us.

All matmul operands are bf16 (walrus forbids mixing f32/f32r with 16-bit):
x, W, Q, K, V1, exp tiles. Accumulation stays fp32 in PSUM; host-measured
rel err ~3.3e-3 against the fp32 reference (tolerance 2e-2). bf16 moving
operands stream 1 col/cycle at any contraction size, so Q/K run unpadded
K=64 and the small diagonal-band matmuls avoid the f32r narrow-moving 4x
penalty.
"""
import numpy as np
import ml_dtypes

import concourse.bass as bass
import concourse.bacc as bacc
import concourse.tile as tile
from concourse import mybir
from concourse.bass_utils import run_bass_kernel_spmd

B, S, DI, DO = 4, 4096, 768, 64
NCORES = 8
NIC = DI // 128          # 6 contraction chunks
NKT = S // 128           # 32 global key tiles per batch
NOWN = NKT // 2          # 16 own key tiles per core
QT = 512                 # query tile width (one PSUM bank of fp32)
NQT = S // QT            # 8 query tiles
F32 = mybir.dt.float32
F32R = mybir.dt.float32r
BF16 = mybir.dt.bfloat16

ORDER = [1, 2, 3, 7, 6, 5, 0, 4]   # arrival-paced; exp-light tiles last
# layout block of each query tile (layout [own0..own3, oth3, oth2, oth1, oth0])
LBLK = {0: 0, 1: 1, 2: 2, 3: 3, 7: 4, 6: 5, 5: 6, 4: 7}
OWNBLK = [0, 1, 2, 3]              # layout block holding own key block st

_prog_cache = {}


def lblk(T):
    """Layout column-block (512 wide) of query tile T."""
    return LBLK[T]


def j0_of(T):
    """First diagonal-region key tile for query tile T."""
    return 4 * T if T < 4 else 4 * (T - 4)


def build_program():
    """Build + compile the single SPMD Bass program (identical on all cores)."""
    nc = bacc.Bacc("TRN2", target_bir_lowering=False, debug=False)

    xT = nc.declare_dram_parameter("xT", [DI, S], BF16, isOutput=False)
    wkv = nc.declare_dram_parameter("wkv", [DI, 128], BF16, isOutput=False)
    # Unpadded Wq: bf16 matmuls stream 1 col/cycle at any contraction
    # size, so scores run K=64 directly.
    wqp = nc.declare_dram_parameter("wqp", [DI, DO], BF16, isOutput=False)
    mdiag = nc.declare_dram_parameter("mdiag", [128, 128], BF16, isOutput=False)
    mpcol = nc.declare_dram_parameter("mpcol", [128, 128], BF16, isOutput=False)
    ident = nc.declare_dram_parameter("ident", [DO, DO], F32R, isOutput=False)
    nd = nc.declare_dram_parameter("nd", [DO + 1, S], F32, isOutput=True)

    with tile.TileContext(nc) as tc:
        with tc.tile_pool(name="consts", bufs=1) as consts, \
             tc.tile_pool(name="xpool", bufs=1) as xpool, \
             tc.tile_pool(name="qkv", bufs=1) as qkv, \
             tc.tile_pool(name="expp", bufs=8) as expp, \
             tc.tile_pool(name="ndst", bufs=2) as ndst, \
             tc.tile_pool(name="ps_proj", bufs=2, space="PSUM") as ps_proj, \
             tc.tile_pool(name="ps_sc2", bufs=2, space="PSUM") as ps_sc2, \
             tc.tile_pool(name="ps_ctx", bufs=2, space="PSUM") as ps_ctx:

            # ---- constants; first projection weights first so PE starts ASAP
            twkv = consts.tile([128, NIC, 128], BF16, tag="twkv", name="twkv")
            twq = consts.tile([128, NIC, DO], BF16, tag="twq", name="twq")
            nc.sync.dma_start(out=twkv, in_=wkv.rearrange("(c p) w -> p c w", p=128))

            # x^T layout blocks: 0 and 1 as [128, 512] singles (fast first
            # arrival), then pairs [128, 1024]; arrival order == consumption.
            xs = [[xpool.tile([128, QT], BF16, tag=f"xs_{ic}_{lb}", name=f"xs_{ic}_{lb}")
                   for lb in range(2)] for ic in range(NIC)]
            xcp = [[xpool.tile([128, 2 * QT], BF16, tag=f"xcp_{ic}_{p}", name=f"xcp_{ic}_{p}")
                    for p in range(1, 4)] for ic in range(NIC)]
            for ic in range(NIC):
                nc.sync.dma_start(out=xs[ic][0],
                                  in_=xT[ic * 128:(ic + 1) * 128, 0:QT])

            twq_dma = nc.sync.dma_start(out=twq,
                                        in_=wqp.rearrange("(c p) w -> p c w", p=128))
            tmd = consts.tile([128, 128], BF16, tag="tmd", name="tmd")
            tmp = consts.tile([128, 128], BF16, tag="tmp", name="tmp")
            tid = consts.tile([DO, DO], F32R, tag="tid", name="tid")
            nc.sync.dma_start(out=tmd, in_=mdiag[:, :])
            nc.sync.dma_start(out=tmp, in_=mpcol[:, :])
            nc.sync.dma_start(out=tid, in_=ident[:, :])
            zsrc = consts.tile([DO, 1], F32, tag="zsrc", name="zsrc")
            nc.vector.memset(zsrc, 0.0)
            ones = consts.tile([128, 1], BF16, tag="ones", name="ones")
            nc.vector.memset(ones, 1.0)

            # Dummy exp to pull the ~2.7us ACT table load off the critical path.
            zexp = consts.tile([DO, 1], F32, tag="zexp", name="zexp")
            nc.scalar.activation(zexp, zsrc,
                                 mybir.ActivationFunctionType.Exp, scale=1.0)

            for ic in range(NIC):
                nc.sync.dma_start(out=xs[ic][1],
                                  in_=xT[ic * 128:(ic + 1) * 128, QT:2 * QT])
            for p in range(1, 4):
                for ic in range(NIC):
                    nc.sync.dma_start(
                        out=xcp[ic][p - 1],
                        in_=xT[ic * 128:(ic + 1) * 128,
                               p * 2 * QT:(p + 1) * 2 * QT])

            def xc(ic, lb):
                """[128, 512] view of layout column block lb."""
                if lb < 2:
                    return xs[ic][lb]
                return xcp[ic][lb // 2 - 1][:, (lb % 2) * QT:(lb % 2 + 1) * QT]

            # ---- projection helpers (emitted interleaved with attention) ----
            kts = [qkv.tile([DO, QT], BF16, tag=f"kt_{st}", name=f"kt_{st}") for st in range(4)]
            vts = [qkv.tile([DO, QT], F32R, tag=f"vt_{st}", name=f"vt_{st}") for st in range(4)]
            qts = [qkv.tile([DO, QT], BF16, tag=f"qt_{st}", name=f"qt_{st}") for st in range(NQT)]
            v1s = [qkv.tile([128, DO + 1], BF16, tag=f"v1_{j}", name=f"v1_{j}")
                   for j in range(NOWN)]

            def emit_pass1(st):
                """[K^T | V^T] matmuls over own key block st + kts/vts casts.

                The V1 transposes are NOT emitted here: they would chain
                PE->DVE->PE->DVE (transpose waits on the vts cast, the next
                scores wait on the v1/qts casts) and park the PE for ~2us per
                block. They are returned as thunks and interleaved into the
                next tile's scores stream, where the PE has slack while ACT
                runs exps.
                """
                p1 = ps_proj.tile([128, QT], F32, tag="psproj", name="psproj")
                for ic in range(NIC):
                    nc.tensor.matmul(p1, twkv[:, ic, :], xc(ic, OWNBLK[st]),
                                     start=(ic == 0), stop=(ic == NIC - 1))
                nc.vector.tensor_copy(kts[st], p1[0:DO, :])
                nc.vector.tensor_copy(vts[st], p1[DO:128, :])

            def transpose_job(j):
                """Build V1[j] ([128 keys, 65] bf16) from vts via PE transpose."""
                st, col = j // 4, (j % 4) * 128
                pv = ps_proj.tile([128, DO], F32R, tag="psproj", name="psproj")
                nc.tensor.transpose(pv, vts[st][:, col:col + 128], tid)
                nc.vector.tensor_copy(v1s[j][:, 0:DO], pv)
                # ones column for the row-sum (denominator)
                nc.vector.tensor_copy(v1s[j][:, DO:DO + 1], ones)

            def emit_pass2(T):
                """Q^T over query tile T (layout block lblk(T)); rows 64.. zero."""
                p2 = ps_proj.tile([DO, QT], F32, tag="psproj", name="psproj")
                for ic in range(NIC):
                    nc.tensor.matmul(p2, twq[:, ic, :], xc(ic, lblk(T)),
                                     start=(ic == 0), stop=(ic == NIC - 1))
                nc.vector.tensor_copy(qts[T], p2)

            # ---- attention: per query tile T, accumulate num/den over key tiles.
            exp_scale = float(1.0 / np.sqrt(DO))

            def emit_scores(T, j, sp_ap):
                """scores matmul for (T, j) into sp_ap ([128, w])."""
                r = j - j0_of(T)
                qlo = 128 * r if r > 0 else 0
                w = QT - qlo
                st, col = j // 4, (j % 4) * 128
                nc.tensor.matmul(sp_ap[:, 0:w], kts[st][:, col:col + 128],
                                 qts[T][:, qlo:QT], start=True, stop=True)
                return qlo, w

            class CtxDrain:
                """Phase B for a query tile, drained a couple of matmuls at a
                time between the NEXT tile's scores pairs so the in-order PE
                queue always holds ready work while ACT runs the exps."""

                def __init__(self, T, ctx_args):
                    self.T = T
                    self.nk = j0_of(T) + 4
                    self.args = ctx_args
                    self.i = 0
                    self.ctxp = ps_ctx.tile([DO + 1, QT], F32, tag="ctxp",
                                            name="ctxp")

                def drain(self, n):
                    while self.i < len(self.args) and n > 0:
                        j, et_ap, qlo, w = self.args[self.i]
                        nc.tensor.matmul(self.ctxp[:, qlo:QT], v1s[j],
                                         et_ap[:, 0:w],
                                         start=(j == 0), stop=(j == self.nk - 1))
                        self.i += 1
                        n -= 1

                def finish(self):
                    self.drain(len(self.args))
                    ost = ndst.tile([DO + 1, QT], F32, tag="ost", name="ost")
                    nc.vector.tensor_copy(ost, self.ctxp)
                    nc.sync.dma_start(
                        out=nd[:, lblk(self.T) * QT:(lblk(self.T) + 1) * QT],
                        in_=ost)

            emit_pass1(0)
            for j in range(4):
                transpose_job(j)   # startup is DMA-bound; no slack lost
            emit_pass1(1)
            emit_pass2(1)
            pending = None           # CtxDrain from the previous iteration
            pending_tr = [4, 5, 6, 7]  # deferred V1 transposes (pass1(1))
            for i, T in enumerate(ORDER):
                j0 = j0_of(T)
                mask = tmd if T < 4 else tmp
                ctx_args = []   # (j, et_ap, qlo, w) consumed in phase B
                for j in range(0, j0, 2):
                    sp2 = ps_sc2.tile([128, 2 * QT], F32, tag="sp2", name="sp2")
                    et2 = expp.tile([128, 2 * QT], BF16, tag="et", name="et")
                    emit_scores(T, j, sp2[:, 0:QT])
                    emit_scores(T, j + 1, sp2[:, QT:2 * QT])
                    nc.scalar.activation(et2, sp2,
                                         mybir.ActivationFunctionType.Exp,
                                         scale=exp_scale)
                    if pending_tr:
                        transpose_job(pending_tr.pop(0))
                    if pending is not None:
                        pending.drain(2)
                    ctx_args.append((j, et2[:, 0:QT], 0, QT))
                    ctx_args.append((j + 1, et2[:, QT:2 * QT], 0, QT))
                # diagonal band: r=0 (w=512) + r=1 (w=384) share a 2-bank tile;
                # r=2 (w=256) + r=3 (w=128) share a 1-bank tile
                spb1 = ps_sc2.tile([128, 2 * QT], F32, tag="sp2", name="sp2")
                etb1 = expp.tile([128, 2 * QT], BF16, tag="et", name="et")
                emit_scores(T, j0, spb1[:, 0:QT])
                emit_scores(T, j0 + 1, spb1[:, QT:QT + 384])
                nc.scalar.activation(etb1[:, 0:QT + 384], spb1[:, 0:QT + 384],
                                     mybir.ActivationFunctionType.Exp,
                                     scale=exp_scale)
                if pending_tr:
                    transpose_job(pending_tr.pop(0))
                if pending is not None:
                    pending.drain(2)
                nc.vector.tensor_mul(etb1[:, 0:128], etb1[:, 0:128], mask)
                nc.vector.tensor_mul(etb1[:, QT:QT + 128], etb1[:, QT:QT + 128], mask)
                ctx_args.append((j0, etb1[:, 0:QT], 0, QT))
                ctx_args.append((j0 + 1, etb1[:, QT:QT + 384], 128, 384))
                spb2 = ps_proj.tile([128, QT], F32, tag="psproj", name="psproj")
                etb2 = expp.tile([128, 2 * QT], BF16, tag="et", name="et")
                emit_scores(T, j0 + 2, spb2[:, 0:256])
                emit_scores(T, j0 + 3, spb2[:, 256:384])
                nc.scalar.activation(etb2[:, 0:384], spb2[:, 0:384],
                                     mybir.ActivationFunctionType.Exp,
                                     scale=exp_scale)
                if pending_tr:
                    transpose_job(pending_tr.pop(0))
                if pending is not None:
                    pending.drain(2)
                nc.vector.tensor_mul(etb2[:, 0:128], etb2[:, 0:128], mask)
                nc.vector.tensor_mul(etb2[:, 256:384], etb2[:, 256:384], mask)
                ctx_args.append((j0 + 2, etb2[:, 0:256], 256, 256))
                ctx_args.append((j0 + 3, etb2[:, 256:384], 384, 128))

                if pending is not None:
                    pending.finish()
                pending = CtxDrain(T, ctx_args)
                if i + 1 < NQT:
                    nxt = ORDER[i + 1]
                    if nxt in (2, 3):   # pass1(0)/pass1(1) emitted in prologue
                        emit_pass1(nxt)
                        pending_tr = list(range(4 * nxt, 4 * nxt + 4))
                    emit_pass2(nxt)
            pending.finish()

    nc.compile()
    return nc


def get_program():
    if "nc" not in _prog_cache:
        _prog_cache["nc"] = build_program()
    return _prog_cache["nc"]


def core_perm(parity):
    """Layout-position -> global column index map.

    Layout = [own0, own1, own2, own3, oth3, oth2, oth1, oth0] 512-col blocks.
    """
    own = [g for g in range(NKT) if g % 2 == parity]
    other = [g for g in range(NKT) if g % 2 != parity]
    tiles = list(own)
    for b in (3, 2, 1, 0):
        tiles += other[4 * b:4 * b + 4]
    return np.concatenate([np.arange(g * 128, (g + 1) * 128) for g in tiles])


def make_in_maps(x, Wq, Wk, Wv):
    x = np.asarray(x, dtype=np.float32)
    Wq = np.asarray(Wq, dtype=np.float32)
    Wk = np.asarray(Wk, dtype=np.float32)
    Wv = np.asarray(Wv, dtype=np.float32)
    bf = ml_dtypes.bfloat16
    wkv = np.concatenate([Wk, Wv], axis=1).astype(bf)
    wqp = Wq.astype(bf)
    mdiag = np.triu(np.ones((128, 128), dtype=np.float32)).astype(bf)
    ident = np.eye(DO, dtype=np.float32)
    in_maps = []
    perms = []
    for c in range(NCORES):
        b, par = c // 2, c % 2
        perm = core_perm(par)
        perms.append(perm)
        xTp = np.ascontiguousarray(x[b].T[:, perm].astype(bf))
        mpcol = np.full((128, 128), 1.0 - par, dtype=np.float32).astype(bf)
        in_maps.append({
            "xT": xTp, "wkv": wkv, "wqp": wqp,
            "mdiag": mdiag, "mpcol": mpcol, "ident": ident,
        })
    return in_maps, perms


def combine(results, perms):
    out = np.empty((B, S, DO), dtype=np.float32)
    for b in range(B):
        num = np.zeros((DO, S), dtype=np.float64)
        den = np.zeros((S,), dtype=np.float64)
        for c in (2 * b, 2 * b + 1):
            nd_c = results[c]["nd"].astype(np.float64)
            inv = np.empty(S, dtype=np.int64)
            inv[perms[c]] = np.arange(S)
            nd_g = nd_c[:, inv]
            num += nd_g[:DO]
            den += nd_g[DO]
        out[b] = (num / den).T.astype(np.float32)
    return out


def kernel(x, Wq, Wk, Wv):
    nc = get_program()
    in_maps, perms = make_in_maps(x, Wq, Wk, Wv)
    res = run_bass_kernel_spmd(nc, in_maps, list(range(NCORES)))
    return combine(res.results, perms)
